# revision 31
# baseline (speedup 1.0000x reference)
"""Trainium2 Bass kernel for a 3x3 stride-1 pad-1 conv, NCHW (16,16,512,512) fp32.

Matches the reference semantics exactly:
  - effective weights: K flattened as (ki,kj,ci) but consumed as (ci,ki,kj):
      Weff[ki,kj,ci,co] = K.reshape(144,16)[ci*9 + ki*3 + kj, co]
  - last output row and column are zero.

Strategy: pure data parallel over the batch (2 images per core on 8 cores),
weights replicated.

DMA-minimal design: the host pre-tiles the (zero-padded, fp16) input into the
exact SBUF layout the matmuls consume (partition = ci*8+hi, free = g*514+col;
group g holds padded rows 6g..6g+7), packed as one fully-contiguous DRAM
block per "macro" of up to 8 groups.  Each macro load is then ONE DMA over a
contiguous ~1MB DRAM window: 128 fat 8KB descriptors spread evenly over all
16 SDMA engines at HBM line rate (a strided staging layout measurably slows
the PE's concurrent SBUF reads -- keep macro blocks contiguous).  Outputs are
stored fp16 the same way (one contiguous [96, ng*512] block per macro) and
unscrambled/upcast on the host.  Macro sizes ramp 2,4,8,... on the first
image and ...,4,2 on the last so the PE starts early and drains fast.

Per group g: K = 8 input rows x 16 c_in = 128 partitions contracted against
banded weights [128, 96] (m = co*6+ho) as 3 fp16 matmuls (one per kj tap)
accumulated in PSUM fp32.  PSUM is drained by fp32->fp16 copies split between
the Vector and Scalar engines into the staging tile, stored once per macro.
"""

import numpy as np

import concourse.bass as bass
import concourse.mybir as mybir
import concourse.tile as tile
from concourse import bacc
from concourse.bass_utils import run_bass_kernel_spmd

F32 = mybir.dt.float32
F16 = mybir.dt.float16

C = 16  # channels (in == out)
W = 512  # image width
H = 512  # image height
R = 6  # output rows per matmul group
RIN = R + 2  # input rows per group
M = R * C  # matmul output partitions (96)
WP = W + 2  # padded row width (514)
HPAD = 518  # padded rows: 1 top + 512 + 5 bottom (group 85 reads rows 510..517)
N_CORES = 8
NG = 86  # groups per image: rows 0..510, group g covers 6g..6g+5


def _macro_schedule(n_img: int) -> list[list[tuple[int, int]]]:
    """Per image: list of (group_start, n_groups). Small macros at the very
    start (fast PE ramp-up) and very end (fast drain)."""
    first = [2, 4] + [8] * 10
    mid = [8] * 10 + [6]
    last = [8] * 9 + [6, 4, 2, 2]
    out = []
    for n in range(n_img):
        if n_img == 1:
            sizes = [2, 4] + [8] * 9 + [6, 2]
        elif n == 0:
            sizes = first
        elif n == n_img - 1:
            sizes = last
        else:
            sizes = mid
        assert sum(sizes) == NG
        starts = np.cumsum([0] + sizes[:-1]).tolist()
        out.append(list(zip(starts, sizes)))
    return out


def _weff(K: np.ndarray) -> np.ndarray:
    Kflat = K.reshape(9 * C, C).astype(np.float32)
    Weff = np.zeros((3, 3, C, C), np.float32)
    for ki in range(3):
        for kj in range(3):
            for ci in range(C):
                Weff[ki, kj, ci, :] = Kflat[ci * 9 + ki * 3 + kj, :]
    return Weff


def _build_weights(K: np.ndarray) -> np.ndarray:
    """lhsT tiled [128, 3, 96] fp16: k = ci*8 + (ho+ki), m = co*6 + ho."""
    Weff = _weff(K)
    wt = np.zeros((3, 128, M), np.float32)
    for ki in range(3):
        for ho in range(R):
            for ci in range(C):
                wt[:, ci * 8 + ho + ki, ho::R] = Weff[ki, :, ci, :]
    return np.ascontiguousarray(wt.transpose(1, 0, 2)).astype(np.float16)


def _tile_input(x16: np.ndarray, sched) -> np.ndarray:
    """[N,16,512,512] fp16 -> packed per-macro-contiguous staging buffer."""
    n_tot = x16.shape[0]
    n_img = len(sched)
    assert n_tot % n_img == 0  # sched is per-core; build per core-chunk
    xp = np.zeros((n_tot, C, HPAD, WP), np.float16)
    xp[:, :, 1:1 + H, 1:1 + W] = x16
    sn, sc, sh, sw = xp.strides
    v = np.lib.stride_tricks.as_strided(
        xp, shape=(n_tot, C, NG, RIN, WP), strides=(sn, sc, 6 * sh, sh, sw))
    # (n, ci, g, hi, col) -> stream (n, ci, hi, g, col) = [n, 128, NG*WP]
    stream = v.transpose(0, 1, 3, 2, 4).reshape(n_tot, 128, NG * WP)
    blocks = []
    for n in range(n_tot):
        for g0, ng in sched[n % n_img]:
            blocks.append(
                np.ascontiguousarray(
                    stream[n, :, g0 * WP:(g0 + ng) * WP]).reshape(-1))
    return np.concatenate(blocks)


def _untile_output(yt_flat: np.ndarray, n_tot: int, sched) -> np.ndarray:
    """packed per-macro fp16 blocks -> [N,16,512,512] fp32 full output."""
    n_img = len(sched)
    stream = np.empty((n_tot, M, NG * W), np.float16)
    off = 0
    for n in range(n_tot):
        for g0, ng in sched[n % n_img]:
            sz = M * ng * W
            stream[n, :, g0 * W:(g0 + ng) * W] = (
                yt_flat[off:off + sz].reshape(M, ng * W))
            off += sz
    # partition = co*6+ho, free = g*512+col; row h = 6g+ho
    rows = stream.reshape(n_tot, C, R, NG, W).transpose(0, 1, 3, 2, 4)
    rows = rows.reshape(n_tot, C, NG * R, W)
    y = np.zeros((n_tot, C, H, W), np.float32)
    y[:, :, :H - 1, :W - 1] = rows[:, :, :H - 1, :W - 1].astype(np.float32)
    return y


def build_nc(n_img: int, in_bufs: int = 6, out_bufs: int = 5,
             psum_bufs: int = 8, warmup: int = 8):
    sched = _macro_schedule(n_img)
    total_in = sum(128 * ng * WP for img in sched for _, ng in img)
    total_out = sum(M * ng * W for img in sched for _, ng in img)

    nc = bacc.Bacc(None, target_bir_lowering=False)
    xs = nc.dram_tensor("xs", [total_in], F16, kind="ExternalInput")
    wt = nc.dram_tensor("wt", [128, 3 * M], F16, kind="ExternalInput")
    ys = nc.dram_tensor("ys", [total_out], F16, kind="ExternalOutput")

    with tile.TileContext(nc) as tc:
        with (
            tc.tile_pool(name="wpool", bufs=1) as wpool,
            tc.tile_pool(name="inpool", bufs=in_bufs) as inpool,
            tc.tile_pool(name="outpool", bufs=out_bufs) as outpool,
            tc.tile_pool(name="psum", bufs=psum_bufs, space="PSUM") as psum_pool,
        ):
            # weights first on the sync ring: they are tiny (74KB) and gate
            # the very first matmul, so they must land before load 0
            wt_t = wpool.tile([128, 3, M], F16)
            nc.sync.dma_start(wt_t[:], bass.AP(wt, 0, [[3 * M, 128],
                                                       [1, 3 * M]]))

            # dummy matmuls on an (uninitialized) scratch tile while load 0
            # is in flight: burns through the HAM 1.2GHz activity window so
            # the first real matmuls already run at 2.4GHz
            if warmup:
                wu_in = wpool.tile([128, W], F16)
                wu_ps = psum_pool.tile([M, W], F32, name="wu", tag="ps")
                nc.vector.memset(wu_in[:], 0.0)
                for _ in range(warmup):
                    nc.tensor.matmul(wu_ps[:], wu_in[:, 0:M], wu_in[:],
                                     start=True, stop=True)

            in_off = 0
            out_off = 0
            for n in range(n_img):
                for im, (g0, ng) in enumerate(sched[n]):
                    t = inpool.tile([128, 8 * WP], F16, name=f"in_{n}_{g0}",
                                    tag="in")
                    src = bass.AP(xs, in_off, [[ng * WP, 128], [1, ng * WP]])
                    nc.sync.dma_start(t[:, 0:ng * WP], src)
                    in_off += 128 * ng * WP

                    ps = [
                        psum_pool.tile([M, W], F32, name=f"ps_{n}_{g0}_{j}",
                                       tag="ps")
                        for j in range(ng)
                    ]
                    # kj-major: each PSUM bank's 3 accumulating writes are
                    # spaced ng MMs apart (back-to-back same-bank accumulation
                    # produces wrong results and runs slower)
                    for kj in range(3):
                        for j in range(ng):
                            nc.tensor.matmul(
                                ps[j][:], wt_t[:, kj, :],
                                t[:, j * WP + kj:j * WP + kj + W],
                                start=(kj == 0), stop=(kj == 2),
                            )

                    out_t = outpool.tile([M, 8 * W], F16,
                                         name=f"out_{n}_{g0}", tag="out")
                    n_dve = max(1, (ng * 5 + 4) // 8)  # ~5/8 of copies on DVE
                    for j in range(ng):
                        dst = out_t[:, j * W:(j + 1) * W]
                        if j < n_dve:
                            nc.vector.tensor_copy(dst, ps[j][:])
                        else:
                            nc.scalar.copy(dst, ps[j][:])
                    dst = bass.AP(ys, out_off, [[ng * W, M], [1, ng * W]])
                    # the last image's final two stores go on the (by then
                    # idle) sync ring so they fire concurrently with the
                    # scalar ring's copy/store tail instead of behind it
                    if n == n_img - 1 and im >= len(sched[n]) - 2:
                        nc.sync.dma_start(dst, out_t[:, 0:ng * W])
                    else:
                        nc.scalar.dma_start(dst, out_t[:, 0:ng * W])
                    out_off += M * ng * W

    nc.finalize()
    return nc


def _run(x: np.ndarray, K: np.ndarray, core_ids, trace=False, **kw):
    """x: [n_total, C, H, W] fp32, split evenly over core_ids."""
    n_cores = len(core_ids)
    n_total = x.shape[0]
    assert n_total % n_cores == 0
    n_per = n_total // n_cores
    sched = _macro_schedule(n_per)
    wt = _build_weights(K)
    nc = build_nc(n_per, **kw)
    per_core_in = sum(128 * ng * WP for img in sched for _, ng in img)
    xs_all = _tile_input(x.astype(np.float16), sched)
    in_maps = [
        {
            "xs": np.ascontiguousarray(
                xs_all[i * per_core_in:(i + 1) * per_core_in]),
            "wt": wt,
        }
        for i in range(n_cores)
    ]
    res = run_bass_kernel_spmd(nc, in_maps, core_ids=list(core_ids),
                               trace=trace)
    yt = np.concatenate([r["ys"] for r in res.results])
    return _untile_output(yt, n_total, sched), res


def kernel(**inputs) -> np.ndarray:
    x = np.ascontiguousarray(np.asarray(inputs["x"], dtype=np.float32))
    K = np.ascontiguousarray(np.asarray(inputs["K"], dtype=np.float32))
    y, _ = _run(x, K, core_ids=range(N_CORES))
    return y


# revision 32
# speedup vs baseline: 1.0858x; 1.0858x over previous
"""Trainium2 Bass kernel for a 3x3 stride-1 pad-1 conv, NCHW (16,16,512,512) fp32.

Matches the reference semantics exactly:
  - effective weights: K flattened as (ki,kj,ci) but consumed as (ci,ki,kj):
      Weff[ki,kj,ci,co] = K.reshape(144,16)[ci*9 + ki*3 + kj, co]
  - last output row and column are zero.

Strategy: pure data parallel over the batch (2 images per core on 8 cores),
weights replicated.

DMA-minimal design: the host pre-tiles the (zero-padded, fp16) input into the
exact SBUF layout the matmuls consume (partition = ci*8+hi, free = g*514+col;
group g holds padded rows 6g..6g+7), packed as one fully-contiguous DRAM
block per "macro" of up to 8 groups.  Each macro load is then ONE DMA over a
contiguous ~1MB DRAM window: 128 fat 8KB descriptors spread evenly over all
16 SDMA engines at HBM line rate (a strided staging layout measurably slows
the PE's concurrent SBUF reads -- keep macro blocks contiguous).  Outputs are
stored fp16 the same way (one contiguous [96, ng*512] block per macro) and
unscrambled/upcast on the host.  Macro sizes ramp 2,4,8,... on the first
image and ...,4,2 on the last so the PE starts early and drains fast.

Per group g: K = 8 input rows x 16 c_in = 128 partitions contracted against
banded weights [128, 96] (m = co*6+ho) as 3 fp16 matmuls (one per kj tap)
accumulated in PSUM fp32.  PSUM is drained by fp32->fp16 copies split between
the Vector and Scalar engines into the staging tile, stored once per macro.
"""

import numpy as np

import concourse.bass as bass
import concourse.mybir as mybir
import concourse.tile as tile
from concourse import bacc
from concourse.bass_utils import run_bass_kernel_spmd

F32 = mybir.dt.float32
F16 = mybir.dt.float16

C = 16  # channels (in == out)
W = 512  # image width
H = 512  # image height
R = 6  # output rows per matmul group
RIN = R + 2  # input rows per group
M = R * C  # matmul output partitions (96)
WP = W + 2  # padded row width (514)
HPAD = 518  # padded rows: 1 top + 512 + 5 bottom (group 85 reads rows 510..517)
N_CORES = 8
NG = 86  # groups per image: rows 0..510, group g covers 6g..6g+5


def _macro_schedule(n_img: int) -> list[list[tuple[int, int]]]:
    """Per image: list of (group_start, n_groups). Small macros at the very
    start (fast PE ramp-up) and very end (fast drain)."""
    first = [2, 4] + [8] * 10
    mid = [8] * 10 + [6]
    last = [8] * 9 + [6, 4, 2, 2]
    out = []
    for n in range(n_img):
        if n_img == 1:
            sizes = [2, 4] + [8] * 9 + [6, 2]
        elif n == 0:
            sizes = first
        elif n == n_img - 1:
            sizes = last
        else:
            sizes = mid
        assert sum(sizes) == NG
        starts = np.cumsum([0] + sizes[:-1]).tolist()
        out.append(list(zip(starts, sizes)))
    return out


def _weff(K: np.ndarray) -> np.ndarray:
    Kflat = K.reshape(9 * C, C).astype(np.float32)
    Weff = np.zeros((3, 3, C, C), np.float32)
    for ki in range(3):
        for kj in range(3):
            for ci in range(C):
                Weff[ki, kj, ci, :] = Kflat[ci * 9 + ki * 3 + kj, :]
    return Weff


def _build_weights(K: np.ndarray) -> np.ndarray:
    """lhsT tiled [128, 3, 96] fp16: k = ci*8 + (ho+ki), m = co*6 + ho."""
    Weff = _weff(K)
    wt = np.zeros((3, 128, M), np.float32)
    for ki in range(3):
        for ho in range(R):
            for ci in range(C):
                wt[:, ci * 8 + ho + ki, ho::R] = Weff[ki, :, ci, :]
    return np.ascontiguousarray(wt.transpose(1, 0, 2)).astype(np.float16)


def _tile_input(x16: np.ndarray, sched) -> np.ndarray:
    """[N,16,512,512] fp16 -> packed per-macro-contiguous staging buffer."""
    n_tot = x16.shape[0]
    n_img = len(sched)
    assert n_tot % n_img == 0  # sched is per-core; build per core-chunk
    xp = np.zeros((n_tot, C, HPAD, WP), np.float16)
    xp[:, :, 1:1 + H, 1:1 + W] = x16
    sn, sc, sh, sw = xp.strides
    v = np.lib.stride_tricks.as_strided(
        xp, shape=(n_tot, C, NG, RIN, WP), strides=(sn, sc, 6 * sh, sh, sw))
    # (n, ci, g, hi, col) -> stream (n, ci, hi, g, col) = [n, 128, NG*WP]
    stream = v.transpose(0, 1, 3, 2, 4).reshape(n_tot, 128, NG * WP)
    blocks = []
    for n in range(n_tot):
        for g0, ng in sched[n % n_img]:
            blocks.append(
                np.ascontiguousarray(
                    stream[n, :, g0 * WP:(g0 + ng) * WP]).reshape(-1))
    return np.concatenate(blocks)


def _untile_output(yt_flat: np.ndarray, n_tot: int, sched) -> np.ndarray:
    """packed per-macro fp16 blocks -> [N,16,512,512] fp32 full output."""
    n_img = len(sched)
    stream = np.empty((n_tot, M, NG * W), np.float16)
    off = 0
    for n in range(n_tot):
        for g0, ng in sched[n % n_img]:
            sz = M * ng * W
            stream[n, :, g0 * W:(g0 + ng) * W] = (
                yt_flat[off:off + sz].reshape(M, ng * W))
            off += sz
    # partition = co*6+ho, free = g*512+col; row h = 6g+ho
    rows = stream.reshape(n_tot, C, R, NG, W).transpose(0, 1, 3, 2, 4)
    rows = rows.reshape(n_tot, C, NG * R, W)
    y = np.zeros((n_tot, C, H, W), np.float32)
    y[:, :, :H - 1, :W - 1] = rows[:, :, :H - 1, :W - 1].astype(np.float32)
    return y


def build_nc(n_img: int, in_bufs: int = 6, out_bufs: int = 5,
             psum_bufs: int = 8, warmup: int = 8):
    sched = _macro_schedule(n_img)
    total_in = sum(128 * ng * WP for img in sched for _, ng in img)
    total_out = sum(M * ng * W for img in sched for _, ng in img)

    nc = bacc.Bacc(None, target_bir_lowering=False)
    xs = nc.dram_tensor("xs", [total_in], F16, kind="ExternalInput")
    wt = nc.dram_tensor("wt", [128, 3 * M], F16, kind="ExternalInput")
    ys = nc.dram_tensor("ys", [total_out], F16, kind="ExternalOutput")

    with tile.TileContext(nc) as tc:
        with (
            tc.tile_pool(name="wpool", bufs=1) as wpool,
            tc.tile_pool(name="inpool", bufs=in_bufs) as inpool,
            tc.tile_pool(name="outpool", bufs=out_bufs) as outpool,
            tc.tile_pool(name="psum", bufs=psum_bufs, space="PSUM") as psum_pool,
        ):
            # weights first on the sync ring: they are tiny (74KB) and gate
            # the very first matmul, so they must land before load 0
            wt_t = wpool.tile([128, 3, M], F16)
            nc.sync.dma_start(wt_t[:], bass.AP(wt, 0, [[3 * M, 128],
                                                       [1, 3 * M]]))

            # dummy matmuls on an (uninitialized) scratch tile while load 0
            # is in flight: burns through the HAM 1.2GHz activity window so
            # the first real matmuls already run at 2.4GHz
            if warmup:
                wu_in = wpool.tile([128, W], F16)
                wu_ps = psum_pool.tile([M, W], F32, name="wu", tag="ps")
                nc.vector.memset(wu_in[:], 0.0)
                for _ in range(warmup):
                    nc.tensor.matmul(wu_ps[:], wu_in[:, 0:M], wu_in[:],
                                     start=True, stop=True)

            in_off = 0
            out_off = 0
            for n in range(n_img):
                for im, (g0, ng) in enumerate(sched[n]):
                    t = inpool.tile([128, 8 * WP], F16, name=f"in_{n}_{g0}",
                                    tag="in")
                    src = bass.AP(xs, in_off, [[ng * WP, 128], [1, ng * WP]])
                    nc.sync.dma_start(t[:, 0:ng * WP], src)
                    in_off += 128 * ng * WP

                    ps = [
                        psum_pool.tile([M, W], F32, name=f"ps_{n}_{g0}_{j}",
                                       tag="ps")
                        for j in range(ng)
                    ]
                    # kj-major: each PSUM bank's 3 accumulating writes are
                    # spaced ng MMs apart (back-to-back same-bank accumulation
                    # produces wrong results and runs slower)
                    for kj in range(3):
                        for j in range(ng):
                            nc.tensor.matmul(
                                ps[j][:], wt_t[:, kj, :],
                                t[:, j * WP + kj:j * WP + kj + W],
                                start=(kj == 0), stop=(kj == 2),
                            )

                    out_t = outpool.tile([M, 8 * W], F16,
                                         name=f"out_{n}_{g0}", tag="out")
                    n_dve = max(1, (ng * 5 + 4) // 8)  # ~5/8 of copies on DVE
                    for j in range(ng):
                        dst = out_t[:, j * W:(j + 1) * W]
                        if j < n_dve:
                            nc.vector.tensor_copy(dst, ps[j][:])
                        else:
                            nc.scalar.copy(dst, ps[j][:])
                    dst = bass.AP(ys, out_off, [[ng * W, M], [1, ng * W]])
                    nc.scalar.dma_start(dst, out_t[:, 0:ng * W])
                    out_off += M * ng * W

    nc.finalize()
    return nc


def _run(x: np.ndarray, K: np.ndarray, core_ids, trace=False, **kw):
    """x: [n_total, C, H, W] fp32, split evenly over core_ids."""
    n_cores = len(core_ids)
    n_total = x.shape[0]
    assert n_total % n_cores == 0
    n_per = n_total // n_cores
    sched = _macro_schedule(n_per)
    wt = _build_weights(K)
    nc = build_nc(n_per, **kw)
    per_core_in = sum(128 * ng * WP for img in sched for _, ng in img)
    xs_all = _tile_input(x.astype(np.float16), sched)
    in_maps = [
        {
            "xs": np.ascontiguousarray(
                xs_all[i * per_core_in:(i + 1) * per_core_in]),
            "wt": wt,
        }
        for i in range(n_cores)
    ]
    res = run_bass_kernel_spmd(nc, in_maps, core_ids=list(core_ids),
                               trace=trace)
    yt = np.concatenate([r["ys"] for r in res.results])
    return _untile_output(yt, n_total, sched), res


def kernel(**inputs) -> np.ndarray:
    x = np.ascontiguousarray(np.asarray(inputs["x"], dtype=np.float32))
    K = np.ascontiguousarray(np.asarray(inputs["K"], dtype=np.float32))
    y, _ = _run(x, K, core_ids=range(N_CORES))
    return y
